# revision 57
# baseline (speedup 1.0000x reference)
"""Causal single-head attention (N=4096, D=F=1024) on 8 TRN2 NeuronCores.

Strided sequence-parallel sharding: core c owns query tiles {c, 8+c,
16+c, 24+c} (128 rows each; "slot" j = the j-th of those tiles). Slot j
only attends to key tiles 0..8(j+1)-1, so every core runs the identical
program while doing 80/128 of the non-causal score work.

Two SPMD launches:
  A) QKV projection over the core's own 512 (strided) rows.
  B) attention + output projection. Keys/values are replicated inputs in
     natural order (no per-core rotation); the causal boundary is applied
     with a per-core multiplicative mask on the first 128 score columns
     of each key tile (the only columns where validity is core-dependent).

Matmul operands are bf16 (f32 PSUM accumulation); all DMAs are blocked so
each descriptor moves >=2KB contiguous per partition.
"""

import sys

try:
    import concourse.bass as bass
except ImportError:  # pragma: no cover
    sys.path.insert(0, "/opt/trn_rl_repo")
    import concourse.bass as bass

import ml_dtypes
import numpy as np

import concourse.mybir as mybir
import concourse.tile as tile
from concourse import bacc
from concourse.bass_utils import run_bass_kernel_spmd

N, D, F = 4096, 1024, 1024
C = 8              # cores
NL = N // C        # 512 query rows per core
P = 128
SCALE = 1.0 / float(np.sqrt(np.float32(F)))

F32 = mybir.dt.float32
MM_DT = mybir.dt.bfloat16

DT = D // P        # 8 contraction tiles
FT = F // P        # 8 f tiles
MT = N // P        # 32 key tiles
NS = 4             # query slots per core (128 rows each)

LAST_EXEC_NS = [None, None]
LAST_RESULTS = [None, None]

_CACHE = {}


def _build_qkv():
    nc = bacc.Bacc(None, target_bir_lowering=False)
    xT = nc.dram_tensor("xT", [P, DT, NL], MM_DT, kind="ExternalInput")
    wqb = nc.dram_tensor("wqb", [FT, P, DT, P], MM_DT, kind="ExternalInput")
    wkb = nc.dram_tensor("wkb", [FT, P, DT, P], MM_DT, kind="ExternalInput")
    wvb = nc.dram_tensor("wvb", [2, P, DT, 512], MM_DT, kind="ExternalInput")
    bq = nc.dram_tensor("bq", [P, FT], F32, kind="ExternalInput")
    bk = nc.dram_tensor("bk", [P, FT], F32, kind="ExternalInput")
    bvB = nc.dram_tensor("bvB", [1, F], F32, kind="ExternalInput")
    qT_o = nc.dram_tensor("qT_o", [F, NL], MM_DT, kind="ExternalOutput")
    kT_o = nc.dram_tensor("kT_o", [F, NL], MM_DT, kind="ExternalOutput")
    v_o = nc.dram_tensor("v_o", [NL, F], MM_DT, kind="ExternalOutput")

    with tile.TileContext(nc) as tc:
        with (
            tc.tile_pool(name="singles", bufs=1) as singles,
            tc.tile_pool(name="weights", bufs=8) as weights,
            tc.tile_pool(name="osb", bufs=6) as opool,
            tc.tile_pool(name="psum", bufs=6, space="PSUM") as psum,
        ):
            warm = singles.tile([P, NL], MM_DT)
            nc.vector.memset(warm, 0.0)
            wps = psum.tile([P, NL], F32, tag="ps")
            for wi in range(16):
                nc.tensor.matmul(
                    wps,
                    warm[:, :P],
                    warm,
                    start=(wi == 0),
                    stop=(wi == 15),
                )
            xT_sb = singles.tile([P, DT, NL], MM_DT)
            nc.sync.dma_start(out=xT_sb[:, : DT // 2, :], in_=xT.ap()[:, : DT // 2, :])
            nc.scalar.dma_start(
                out=xT_sb[:, DT // 2 :, :], in_=xT.ap()[:, DT // 2 :, :]
            )
            bq_sb = singles.tile([P, FT], F32)
            nc.gpsimd.dma_start(out=bq_sb, in_=bq.ap())
            bk_sb = singles.tile([P, FT], F32)
            nc.gpsimd.dma_start(out=bk_sb, in_=bk.ap())
            bvB_sb = singles.tile([P, F], F32)
            nc.gpsimd.dma_start(out=bvB_sb, in_=bvB.ap().to_broadcast([P, F]))

            # q.T / k.T : out[f_tile, n] = sum_d wT[d, f] * xT[d, n]
            for wi_, (w_t, b_sb, out_t) in enumerate(
                ((wqb, bq_sb, qT_o), (wkb, bk_sb, kT_o))
            ):
                for ft in range(FT):
                    wc = weights.tile([P, DT, P], MM_DT, tag="wc")
                    (nc.sync if (ft + wi_) % 2 == 0 else nc.scalar).dma_start(
                        out=wc, in_=w_t.ap()[ft]
                    )
                    ps = psum.tile([P, NL], F32, tag="ps")
                    for dt_i in range(DT):
                        nc.tensor.matmul(
                            ps,
                            wc[:, dt_i, :],
                            xT_sb[:, dt_i, :],
                            start=(dt_i == 0),
                            stop=(dt_i == DT - 1),
                        )
                    osb = opool.tile([P, NL], MM_DT, tag="osb")
                    nc.vector.tensor_scalar_add(
                        out=osb, in0=ps, scalar1=b_sb[:, ft : ft + 1]
                    )
                    (nc.scalar if ft % 2 == 0 else nc.sync).dma_start(
                        out=out_t.ap()[ft * P : (ft + 1) * P, :], in_=osb
                    )

            # v : out[m_tile, f] = sum_d xT[d, m] * wvT[d, f]
            for fc in range(2):
                fs = slice(fc * 512, (fc + 1) * 512)
                wvc = weights.tile([P, DT, 512], MM_DT, tag="wvc")
                (nc.sync if fc == 0 else nc.scalar).dma_start(
                    out=wvc, in_=wvb.ap()[fc]
                )
                for mi in range(NS):
                    ps = psum.tile([P, 512], F32, tag="ps")
                    for dt_i in range(DT):
                        nc.tensor.matmul(
                            ps,
                            xT_sb[:, dt_i, mi * P : (mi + 1) * P],
                            wvc[:, dt_i, :],
                            start=(dt_i == 0),
                            stop=(dt_i == DT - 1),
                        )
                    vsb = opool.tile([P, 512], MM_DT, tag="osb")
                    nc.vector.tensor_add(out=vsb, in0=ps, in1=bvB_sb[:, fs])
                    (nc.scalar if mi % 2 == 0 else nc.sync).dma_start(
                        out=v_o.ap()[mi * P : (mi + 1) * P, fs], in_=vsb
                    )
    nc.finalize()
    return nc


def _build_attn():
    nc = bacc.Bacc(None, target_bir_lowering=False)
    qT = nc.dram_tensor("qT", [P, FT, NL], MM_DT, kind="ExternalInput")
    kbs = nc.dram_tensor("kbs", [MT, P, FT, P], MM_DT, kind="ExternalInput")
    # vbk[ft] = [128 keys-part, 32 m, 128 f] (k-major so DMA is linear)
    vbk = nc.dram_tensor("vbk", [FT, P, MT, P], MM_DT, kind="ExternalInput")
    mask = nc.dram_tensor("mask", [P, MT, P], mybir.dt.float8e4, kind="ExternalInput")
    projT = nc.dram_tensor("projT", [F, F], MM_DT, kind="ExternalInput")
    pbB = nc.dram_tensor("pbB", [1, F], F32, kind="ExternalInput")
    out_o = nc.dram_tensor("out_o", [NL, F], MM_DT, kind="ExternalOutput")

    def wof(m):  # score-tile width and column offset for key tile m
        j0 = m // 8
        return NL - P * j0, P * j0

    with tile.TileContext(nc) as tc:
        with (
            tc.tile_pool(name="singles", bufs=1) as singles,
            tc.tile_pool(name="kc", bufs=20) as kpool,
            tc.tile_pool(name="pt", bufs=MT) as ptpool,
            tc.tile_pool(name="vc", bufs=5) as vpool,
            tc.tile_pool(name="osb", bufs=3) as opool,
            tc.tile_pool(name="sps", bufs=3, space="PSUM") as spsum,
            tc.tile_pool(name="rps", bufs=1, space="PSUM") as rpsum,
            tc.tile_pool(name="zps", bufs=3, space="PSUM") as zpsum,
            tc.tile_pool(name="dram", bufs=1, space="DRAM") as drampool,
        ):
            warm = singles.tile([P, NL], MM_DT)
            nc.vector.memset(warm, 0.0)
            qq = [nc.sync, nc.scalar, nc.gpsimd]
            # first tile's keys lead the sync queue so scores start early
            kc0 = kpool.tile([P, FT, P], MM_DT, tag="kc")
            nc.sync.dma_start(out=kc0, in_=kbs.ap()[0])
            qT_sb = singles.tile([P, FT, NL], MM_DT)
            nc.scalar.dma_start(out=qT_sb[:, :2, :], in_=qT.ap()[:, :2, :])
            nc.scalar.dma_start(out=qT_sb[:, 2:4, :], in_=qT.ap()[:, 2:4, :])
            nc.sync.dma_start(out=qT_sb[:, 4:6, :], in_=qT.ap()[:, 4:6, :])
            nc.sync.dma_start(out=qT_sb[:, 6:, :], in_=qT.ap()[:, 6:, :])
            wps = spsum.tile([P, NL], F32, tag="sps")
            for wi in range(16):
                nc.tensor.matmul(
                    wps,
                    warm[:, :P],
                    warm,
                    start=(wi == 0),
                    stop=(wi == 15),
                )
            mask_sb = singles.tile([P, MT, P], mybir.dt.float8e4)
            nc.gpsimd.dma_start(out=mask_sb, in_=mask.ap())
            ones_sb = singles.tile([P, 1], MM_DT)
            nc.vector.memset(ones_sb, 1.0)

            # gpsimd (slow software DGE, otherwise idle) prefetches the
            # first halves of the early z-phase v tiles during scores
            vcs = [None] * FT
            for ft in range(5):
                vc_pre = vpool.tile([P, MT, P], MM_DT, tag="vc", name=f"vc{ft}")
                nc.gpsimd.dma_start(
                    out=vc_pre[:, : MT // 2, :],
                    in_=vbk.ap()[ft, :, : MT // 2, :],
                )
                vcs[ft] = vc_pre

            # ---- scores + exp + causal mask on first 128 columns
            # kc tiles only on the two hardware DGE queues
            pts = [None] * MT
            for m in range(MT):
                W, off = wof(m)
                if m == 0:
                    kc = kc0
                else:
                    kc = kpool.tile([P, FT, P], MM_DT, tag="kc")
                    qq[m % 2].dma_start(out=kc, in_=kbs.ap()[m])
                ps = spsum.tile([P, NL], F32, tag="sps")
                for ft in range(FT):
                    nc.tensor.matmul(
                        ps[:, :W],
                        kc[:, ft, :],
                        qT_sb[:, ft, off:],
                        start=(ft == 0),
                        stop=(ft == FT - 1),
                    )
                pt = ptpool.tile([P, W], MM_DT, tag="pt")
                nc.scalar.activation(
                    out=pt,
                    in_=ps[:, :W],
                    func=mybir.ActivationFunctionType.Exp,
                    scale=SCALE,
                )
                nc.vector.tensor_mul(
                    out=pt[:, :P], in0=pt[:, :P], in1=mask_sb[:, m, :]
                )
                pts[m] = pt

            # ---- row sums: one [1, NL] psum; tile m only touches the
            # columns of slots j >= m//8, which is exactly the valid set.
            rps = rpsum.tile([1, NL], F32)
            for m in range(MT):
                W, off = wof(m)
                nc.tensor.matmul(
                    rps[:, off:],
                    ones_sb,
                    pts[m],
                    start=(m == 0),
                    stop=(m == MT - 1),
                )
            recip_row = singles.tile([1, NL], F32)
            nc.vector.reciprocal(out=recip_row, in_=rps)
            scratch = drampool.tile([1, NL], F32)
            nc.gpsimd.dma_start(out=scratch, in_=recip_row)
            recip_np = singles.tile([P, NS], F32)
            nc.gpsimd.dma_start(
                out=recip_np, in_=scratch[0].rearrange("(t p) -> p t", p=P)
            )

            # ---- z.T[f, q] = sum_m v[m][k, f] * pt_m[k, q]
            pbB_sb = singles.tile([P, F], F32)
            projT_sb = singles.tile([P, FT, F], MM_DT)
            nc.scalar.dma_start(
                out=pbB_sb, in_=pbB.ap().to_broadcast([P, F])
            )
            nc.scalar.dma_start(
                out=projT_sb,
                in_=projT.ap().rearrange("(t p) f -> p t f", p=P),
            )
            zsb = singles.tile([P, FT, NL], MM_DT)
            for ft in range(FT):
                if vcs[ft] is not None:
                    # first half already prefetched on gpsimd during scores
                    vc = vcs[ft]
                    (nc.sync if ft % 2 == 0 else nc.scalar).dma_start(
                        out=vc[:, MT // 2 :, :],
                        in_=vbk.ap()[ft, :, MT // 2 :, :],
                    )
                else:
                    vc = vpool.tile([P, MT, P], MM_DT, tag="vc")
                    nc.sync.dma_start(
                        out=vc[:, : MT // 2, :],
                        in_=vbk.ap()[ft, :, : MT // 2, :],
                    )
                    nc.scalar.dma_start(
                        out=vc[:, MT // 2 :, :],
                        in_=vbk.ap()[ft, :, MT // 2 :, :],
                    )
                zps = zpsum.tile([P, NL], F32, tag="zps")
                for m in range(MT):
                    W, off = wof(m)
                    nc.tensor.matmul(
                        zps[:, off:],
                        vc[:, m, :],
                        pts[m],
                        start=(m == 0),
                        stop=(m == MT - 1),
                    )
                nc.vector.tensor_copy(out=zsb[:, ft, :], in_=zps)

            # ---- out[q, o] = (z.T/rowsum) @ projT + pb, per slot
            for j in range(NS):
                for oc in range(2):
                    os_ = slice(oc * 512, (oc + 1) * 512)
                    ops = zpsum.tile([P, 512], F32, tag="zps")
                    for ft in range(FT):
                        nc.tensor.matmul(
                            ops,
                            zsb[:, ft, j * P : (j + 1) * P],
                            projT_sb[:, ft, os_],
                            start=(ft == 0),
                            stop=(ft == FT - 1),
                        )
                    osb = opool.tile([P, 512], MM_DT, tag="osb")
                    nc.vector.scalar_tensor_tensor(
                        out=osb,
                        in0=ops,
                        scalar=recip_np[:, j : j + 1],
                        in1=pbB_sb[:, os_],
                        op0=mybir.AluOpType.mult,
                        op1=mybir.AluOpType.add,
                    )
                    (nc.sync if oc == 0 else nc.scalar).dma_start(
                        out=out_o.ap()[j * P : (j + 1) * P, os_], in_=osb
                    )
    nc.finalize()
    return nc


def _get_programs():
    if "qkv" not in _CACHE:
        _CACHE["qkv"] = _build_qkv()
        _CACHE["attn"] = _build_attn()
    return _CACHE["qkv"], _CACHE["attn"]


def _c(a):
    return np.ascontiguousarray(a, dtype=np.float32)


def _b(a):
    return np.ascontiguousarray(np.asarray(a, dtype=np.float32).astype(ml_dtypes.bfloat16))


def kernel(x, wq_w, wq_b, wk_w, wk_b, wv_w, wv_b, proj_w, proj_b):
    x = np.asarray(x, dtype=np.float32)
    nc_qkv, nc_attn = _get_programs()

    # ---- launch A: QKV projection; core c owns query tiles {c, 8+c, 16+c, 24+c}
    xT = x.T                                     # [D, N]
    xT_tiles = xT.reshape(D, MT, P)              # [D, m, p]
    wqb = _b(np.asarray(wq_w).T.reshape(DT, P, FT, P).transpose(2, 1, 0, 3))
    wkb = _b(np.asarray(wk_w).T.reshape(DT, P, FT, P).transpose(2, 1, 0, 3))
    wvb = _b(np.asarray(wv_w).T.reshape(DT, P, 2, 512).transpose(2, 1, 0, 3))
    bq_pb = _c(np.asarray(wq_b).reshape(FT, P).T)
    bk_pb = _c(np.asarray(wk_b).reshape(FT, P).T)
    bvB = _c(np.asarray(wv_b).reshape(1, F))
    in_a = []
    for c in range(C):
        cols = xT_tiles[:, c::C, :].reshape(D, NL)   # slot-major columns
        xT_blk = _b(cols.reshape(DT, P, NL).transpose(1, 0, 2))
        in_a.append(
            {
                "xT": xT_blk,
                "wqb": wqb,
                "wkb": wkb,
                "wvb": wvb,
                "bq": bq_pb,
                "bk": bk_pb,
                "bvB": bvB,
            }
        )
    res_a = run_bass_kernel_spmd(nc_qkv, in_a, core_ids=list(range(C)))
    LAST_EXEC_NS[0] = res_a.exec_time_ns
    LAST_RESULTS[0] = res_a

    # reassemble full kT [F, N] and v [N, F] from strided shards
    kT_full = np.empty((F, N), dtype=ml_dtypes.bfloat16)
    v_full = np.empty((N, F), dtype=ml_dtypes.bfloat16)
    for c in range(C):
        kt = res_a.results[c]["kT_o"].reshape(F, NS, P)
        vt = res_a.results[c]["v_o"].reshape(NS, P, F)
        for j in range(NS):
            t = 8 * j + c
            kT_full[:, t * P : (t + 1) * P] = kt[:, j, :]
            v_full[t * P : (t + 1) * P, :] = vt[j]

    # ---- launch B: attention + projection
    kbs = np.ascontiguousarray(
        kT_full.reshape(FT, P, MT, P).transpose(2, 1, 0, 3)
    )
    vbk = np.ascontiguousarray(
        v_full.reshape(MT, P, FT, P).transpose(2, 1, 0, 3)
    )
    projT = _b(np.asarray(proj_w).T)
    pbB = _c(np.asarray(proj_b).reshape(1, F))
    # pt layout is [key, query]: valid = key <= query = upper triangular
    f8 = ml_dtypes.float8_e4m3
    tri = np.triu(np.ones((P, P), dtype=np.float32)).astype(f8)
    onesm = np.ones((P, P), dtype=f8)
    zerosm = np.zeros((P, P), dtype=f8)
    in_b = []
    for c in range(C):
        qT_blk = np.ascontiguousarray(
            res_a.results[c]["qT_o"].reshape(FT, P, NL).transpose(1, 0, 2)
        )
        mk = np.empty((MT, P, P), dtype=f8)
        for m in range(MT):
            p = m % 8
            mk[m] = onesm if p < c else (tri if p == c else zerosm)
        mk = np.ascontiguousarray(mk.transpose(1, 0, 2))  # [P, MT, P]
        in_b.append(
            {
                "qT": qT_blk,
                "kbs": kbs,
                "vbk": vbk,
                "mask": mk,
                "projT": projT,
                "pbB": pbB,
            }
        )
    res_b = run_bass_kernel_spmd(nc_attn, in_b, core_ids=list(range(C)))
    LAST_EXEC_NS[1] = res_b.exec_time_ns
    LAST_RESULTS[1] = res_b

    out = np.empty((N, F), dtype=np.float32)
    for c in range(C):
        ob = np.asarray(res_b.results[c]["out_o"], dtype=np.float32).reshape(
            NS, P, F
        )
        for j in range(NS):
            t = 8 * j + c
            out[t * P : (t + 1) * P, :] = ob[j]
    return out
